# revision 6
# baseline (speedup 1.0000x reference)
"""Trainium2 Bass kernel for nn_ASPP (gnn_message_passing).

Strategy (8 NeuronCores, data-parallel over output points):
 - core c owns output rows [c*12500, (c+1)*12500)
 - phase 1: 5 dilated sparse convs: dma_gather (transpose, bf16 hi|lo|hi
   split for ~fp32 precision) -> banded PE matmuls -> DVE 32x32 stream
   transpose -> dma_scatter_add into doubled DRAM tables (occurrence-pair
   rounds make destinations unique per call; HW CCE RMW races otherwise)
 - pooling via one-hot matmul + AllReduce; linear branch dense on PE
 - cat assembled in bf16, AllGather -> phase 2 (output conv) with the
   same gather/GEMM/scatter machinery (contraction 224 = 128 + 96)
 - BN + ReLU on DVE, fold doubled tables.

Wire-size optimization: the axon PJRT path re-ships every input buffer per
execution (~10 GB/s + ~0.5 ms/buffer), so inputs are minimized to 4 compact
tensors per core (~6.5 MB): the x shard (+one-hot batch cols), a [16, W]
un-replicated index stream pack, and two packed weight tensors. A device
prolog AllGathers x, builds the 256B-row gather table (hi|lo|hi bf16),
expands the index pack to the 128-partition layout dma_gather needs, and
derives xT / bh_T via PE transposes.

All index preprocessing is host-side numpy; the SPMD program is shared by
all 8 cores, so per-(group) lengths are uniformized to max-over-cores.
"""
import numpy as np
import ml_dtypes
from contextlib import ExitStack

import concourse.bacc as bacc
import concourse.tile as tile
from concourse import mybir
from concourse.bass_utils import run_bass_kernel_spmd

# problem constants (hardcoded per harness contract)
N = 100000
C = 32
KK = 27
M = 30000
WID = 5
B = 4
EPS = 1e-5
NCORE = 8
S = N // NCORE          # 12500 rows per core
SPAD = 12544            # 98*128
NCHUNK = SPAD // 128    # 98
NG = NCORE * SPAD       # 100352 rows in the padded global row space
TROWS = 25344           # 99*256, >= 2*S doubled table + trash
# pairs n in [0, SPAD) read rows 2n,2n+1 < 25088; trash must be >= 25088
TRASH = 25120
SEG = 6144              # tokens per segment (3 quads)
QUAD = 2048
BUCK = 32768            # int16 gather bucket size
F32 = mybir.dt.float32
BF16 = mybir.dt.bfloat16
I16 = mybir.dt.int16

# bisection flags (timing experiments)
SKIP_GATHER1 = False
SKIP_MM1 = False
SKIP_SCATTER1 = False
TC1_ONLY = False


def _wrap16(vals):
    """[L] -> [16, L//16] int16 wrapped-in-16 (token t at [t%16, t//16])."""
    L = len(vals)
    assert L % 16 == 0
    return np.asarray(vals, np.int16).reshape(-1, 16).T  # [16, L//16]


def _hi_lo(a):
    hi = a.astype(ml_dtypes.bfloat16)
    lo = (a - hi.astype(np.float32)).astype(ml_dtypes.bfloat16)
    return hi, lo


def _prep_streams(in_maps, out_maps, conv_ids, n_src_rows, remap_in=None):
    """Build uniform token streams for a list of convs.

    Returns (meta, percore) where meta has global (uniform) layout info and
    percore[c] has gidx/sidx value streams per (conv, bucket).
    """
    nconv = len(conv_ids)
    # per (core, conv, bucket): edge arrays
    edges = [[[None] * 4 for _ in range(nconv)] for _ in range(NCORE)]
    for ji, j in enumerate(conv_ids):
        im = in_maps[j].reshape(-1).astype(np.int64)   # [27*M]
        om = out_maps[j].reshape(-1).astype(np.int64)
        ke = np.repeat(np.arange(KK, dtype=np.int64), M)
        src = im if remap_in is None else remap_in[im]
        bk = src // BUCK
        core = om // S
        out_local = om - core * S
        for c in range(NCORE):
            mc = core == c
            kc, sc, oc, bc = ke[mc], src[mc], out_local[mc], bk[mc]
            # occurrence rank of out_local within (conv, bucket) subsets
            for b in range(4):
                mb = bc == b
                kb, sb, ob = kc[mb], sc[mb], oc[mb]
                order = np.argsort(ob, kind="stable")
                inv = np.empty_like(order)
                inv[order] = np.arange(len(order))
                obs = ob[order]
                # cumcount within runs of equal out
                is_new = np.ones(len(obs), bool)
                is_new[1:] = obs[1:] != obs[:-1]
                run_id = np.cumsum(is_new) - 1
                starts = np.flatnonzero(is_new)
                occ_sorted = np.arange(len(obs)) - starts[run_id]
                occ = occ_sorted[inv]
                edges[c][ji][b] = (kb, sb, ob, occ)

    # uniform piece lengths L[ji][b][r][k] = max over cores
    meta = {"L": {}, "RMAX": {}, "TOK": {}, "windows": {}, "pieces": {},
            "gcalls": {}, "scalls": {}, "nconv": nconv}
    for ji in range(nconv):
        for b in range(4):
            rmax = 0
            for c in range(NCORE):
                occ = edges[c][ji][b][3]
                if len(occ):
                    rmax = max(rmax, int(occ.max()) // 2 + 1)
            cnts = np.zeros((NCORE, rmax, KK), np.int64)
            for c in range(NCORE):
                kb, sb, ob, occ = edges[c][ji][b]
                r = occ // 2
                np.add.at(cnts, (c, r, kb), 1)
            L = cnts.max(axis=0)  # [rmax, 27]
            meta["L"][(ji, b)] = L
            meta["RMAX"][(ji, b)] = rmax
            # walk layout: token positions. Pair-rounds pad to full quads
            # (2048) so scatter-call windows stay contiguous in the
            # transposed (scatter-order) index space.
            windows = []
            pieces = []   # (seg, quad, band, col0, ts0, length, k)
            pos = 0
            for r in range(rmax):
                wstart = pos
                for k in range(KK):
                    ln = int(L[r, k])
                    if ln == 0:
                        continue
                    # split piece at 512 grid
                    t0 = pos
                    while t0 < pos + ln:
                        t1 = min((t0 // 512 + 1) * 512, pos + ln)
                        seg = t0 // SEG
                        ts = t0 % SEG
                        pieces.append((seg, ts // QUAD, (ts % QUAD) // 512,
                                       ts % 512, ts, t1 - t0, k))
                        t0 = t1
                    pos += ln
                pos = (pos + QUAD - 1) // QUAD * QUAD
                windows.append((wstart, pos))
            meta["TOK"][(ji, b)] = pos
            meta["windows"][(ji, b)] = windows
            meta["pieces"][(ji, b)] = pieces
            # gather calls per segment
            gcalls = []
            for sgs in range(0, pos, SEG):
                gcalls.append((sgs, min(SEG, pos - sgs)))
            meta["gcalls"][(ji, b)] = gcalls
            # scatter calls: windows split at segment boundaries
            scalls = []
            for (a, e) in windows:
                t = a
                while t < e:
                    t1 = min((t // SEG + 1) * SEG, e)
                    scalls.append((t, t1 - t))
                    t = t1
            meta["scalls"][(ji, b)] = scalls

    # group start positions (global layout), vectorized per-core placement
    meta["gstart"] = {}
    for ji in range(nconv):
        for b in range(4):
            L = meta["L"][(ji, b)]
            rmax = meta["RMAX"][(ji, b)]
            gstart = np.zeros((max(rmax, 1), KK), np.int64)
            pos = 0
            for rr in range(rmax):
                for k in range(KK):
                    gstart[rr, k] = pos
                    pos += int(L[rr, k])
                pos = (pos + QUAD - 1) // QUAD * QUAD
            meta["gstart"][(ji, b)] = gstart

    percore = []
    for c in range(NCORE):
        streams = {}
        for ji in range(nconv):
            for b in range(4):
                L = meta["L"][(ji, b)]
                tok = meta["TOK"][(ji, b)]
                gstart = meta["gstart"][(ji, b)]
                kb, sb, ob, occ = edges[c][ji][b]
                gv = np.zeros(tok, np.int16)
                sv = np.full(tok, TRASH, np.int16)
                if len(kb):
                    r = occ // 2
                    okey = r * KK + kb
                    order = np.argsort(okey, kind="stable")
                    kb2, sb2, ob2, r2 = (kb[order], sb[order], ob[order],
                                         r[order])
                    p2 = (occ & 1)[order]
                    okey2 = okey[order]
                    is_new = np.ones(len(okey2), bool)
                    is_new[1:] = okey2[1:] != okey2[:-1]
                    run_id = np.cumsum(is_new) - 1
                    starts = np.flatnonzero(is_new)
                    rank = np.arange(len(okey2)) - starts[run_id]
                    dest = gstart[r2, kb2] + rank
                    gv[dest] = (sb2 - b * BUCK).astype(np.int16)
                    # scatter reads tokens in transposed (i-space) order:
                    # within a quad, t=(512*band+col) sits at scatter slot
                    # i = 128*(col//32) + 32*band + col%32
                    u = dest // QUAD
                    tl = dest % QUAD
                    di = (u * QUAD + 128 * ((tl % 512) // 32)
                          + 32 * (tl // 512) + (tl % 32))
                    sv[di] = (ob2 * 2 + p2).astype(np.int16)
                streams[(ji, b)] = (gv, sv)
        percore.append(streams)
    return meta, percore


def _pack_idx(metas_percore):
    """Concat all streams' per-call wrapped idx blocks into one [16, W]
    array per core.  metas_percore = [(meta, percore), ...] in order.
    Returns (arrs[c], offs_list) where offs_list[i] = (offs_g, offs_s) maps
    (key, call_start) -> (absolute column offset, token length)."""
    col = 0
    offs_list = []
    for meta, _ in metas_percore:
        offs_g, offs_s = {}, {}
        for key in sorted(meta["gcalls"].keys()):
            for (a, ln) in meta["gcalls"][key]:
                offs_g[(key, a)] = (col, ln)
                col += ln // 16
        for key in sorted(meta["gcalls"].keys()):
            for (a, ln) in meta["scalls"][key]:
                offs_s[(key, a)] = (col, ln)
                col += ln // 16
        offs_list.append((offs_g, offs_s))
    wall = col
    arrs = []
    for c in range(NCORE):
        cols = np.zeros((16, wall), np.int16)
        for (meta, percore), (offs_g, offs_s) in zip(metas_percore, offs_list):
            for key in sorted(meta["gcalls"].keys()):
                gv, sv = percore[c][key]
                for (a, ln) in meta["gcalls"][key]:
                    o, _ = offs_g[(key, a)]
                    cols[:, o:o + ln // 16] = _wrap16(gv[a:a + ln])
                for (a, ln) in meta["scalls"][key]:
                    o, _ = offs_s[(key, a)]
                    cols[:, o:o + ln // 16] = _wrap16(sv[a:a + ln])
        arrs.append(cols)
    return arrs, offs_list, wall


def _build(meta1, meta2, wall, offs, split=True):
    offs_g1, offs_s1, offs_g2, offs_s2 = offs
    nc = bacc.Bacc(trn_type="TRN2", target_bir_lowering=False, debug=False,
                   num_devices=NCORE)
    # ---- dram inputs (minimal wire footprint; see module docstring) ----
    xsh_h = nc.dram_tensor("xsh", [SPAD, 36], F32, kind="ExternalInput")
    xsh = xsh_h.ap()                       # cols 0:32 x shard, 32:36 bh_rm
    idxc = nc.dram_tensor("idxc", [16, wall], I16, kind="ExternalInput").ap()
    wbf = nc.dram_tensor("wbf", [128, 6048], BF16, kind="ExternalInput").ap()
    # wf32 cols: 0:32 biasb, 32:64 bnscale, 64:96 bnshift, 96 invc(rows 0:4),
    # 100:132 wlin (rows 0:32)
    wf32 = nc.dram_tensor("wf32", [128, 132], F32, kind="ExternalInput").ap()

    # ---- internal dram ----
    xg_h = nc.dram_tensor("xg", [NG, 36], F32, addr_space="Shared")
    xg = xg_h.ap()
    x_pad = nc.dram_tensor("x_pad", [NG, 128], BF16).ap()
    idxx = nc.dram_tensor("idxx", [128, wall], I16).ap()
    xT_int = nc.dram_tensor("xT_int", [32, SPAD], F32).ap()
    bhT_int = nc.dram_tensor("bhT_int", [4, SPAD], F32).ap()

    # ---- scatter accumulators: internal DRAM, zeroed on device ----
    btabs = [nc.dram_tensor(f"btab{j}", [TROWS, 64], F32).ap()
             for j in range(WID)]
    ytab = nc.dram_tensor("ytab", [TROWS, 64], F32).ap()
    y_out = nc.dram_tensor("y_out", [SPAD, 32], F32, kind="ExternalOutput").ap()

    sums_local = nc.dram_tensor("sums_local", [4, 32], F32)
    sums_red = nc.dram_tensor("sums_red", [4, 32], F32, addr_space="Shared")
    cat_local = nc.dram_tensor("cat_local", [SPAD, 256], BF16)
    catg = nc.dram_tensor("catg", [NG, 256], BF16, addr_space="Shared")
    flush_a = nc.dram_tensor("flush_a", [128, 128], F32)
    flush_b = nc.dram_tensor("flush_b", [128, 128], F32)

    cc_sem = nc.alloc_semaphore("cc_sem")
    fl_id = [0]

    def _flush(sp=True, swdge=True):
        """Ring-flush: HWDGE(SP) and/or SWDGE(q0) are FIFO per ring, so a
        small DMA + sem-wait guarantees all prior DMAs on the ring landed."""
        fl_id[0] += 1
        if sp:
            s1 = nc.alloc_semaphore(f"flsp{fl_id[0]}")
            nc.sync.dma_start(flush_b[:, :], flush_a[:, :]).then_inc(s1, 16)
            nc.gpsimd.wait_ge(s1, 16)
        if swdge:
            s2 = nc.alloc_semaphore(f"flsw{fl_id[0]}")
            nc.gpsimd.dma_start(flush_b[:, :], flush_a[:, :]).then_inc(s2, 16)
            nc.gpsimd.wait_ge(s2, 16)
        nc.all_engine_barrier()

    # ---- start the x AllGather early (collectives cannot read IO tensors,
    # so stage the shard into internal DRAM first) ----
    xloc_h = nc.dram_tensor("xloc", [SPAD, 36], F32)
    xcp_sem = nc.alloc_semaphore("xcp_sem")
    nc.sync.dma_start(xloc_h.ap()[:, :], xsh[:, :]).then_inc(xcp_sem, 16)
    nc.gpsimd.wait_ge(xcp_sem, 16)
    nc.gpsimd.collective_compute(
        "AllGather", mybir.AluOpType.bypass,
        replica_groups=[list(range(NCORE))],
        ins=[xloc_h.ap()], outs=[xg_h.ap()],
    ).then_inc(cc_sem)

    # ===== TC0: zero scatter accumulators + expand idx pack (overlaps the
    # AllGather; none of this depends on xg) =====
    with ExitStack() as ctx:
        tc = ctx.enter_context(tile.TileContext(nc))
        zp = ctx.enter_context(tc.tile_pool(name="zp", bufs=1))
        zt = zp.tile([128, 1024], F32)
        nc.vector.memset(zt[:], 0)
        NA = TROWS // 128  # 198
        for tab in (*btabs, ytab):
            tv = tab.rearrange("(a p) f -> p a f", p=128)
            for a0 in range(0, NA, 16):
                cnt = min(16, NA - a0)
                nc.sync.dma_start(tv[:, a0:a0 + cnt, :], zt[:, 0:cnt * 64])
        # idx expansion: replicate [16, wall] into the 8 16-partition groups
        for g in range(8):
            nc.sync.dma_start(idxx[16 * g:16 * g + 16, :], idxc[:, :])
    _flush(sp=True, swdge=False)
    nc.gpsimd.wait_ge(cc_sem, 1)
    nc.all_engine_barrier()

    # ===== TCP: prolog — build x_pad (hi|lo|hi bf16) from xg; derive
    # xT_int / bhT_int from the local shard via PE transposes =====
    with ExitStack() as ctx:
        tc = ctx.enter_context(tile.TileContext(nc))
        ppool = ctx.enter_context(tc.tile_pool(name="pp", bufs=3))
        idp = ctx.enter_context(tc.tile_pool(name="idp", bufs=1))
        psp = ctx.enter_context(tc.tile_pool(name="psp", bufs=4, space="PSUM"))
        GP = 16
        NGC = NG // 128  # 784
        xg_v = xg.rearrange("(a p) c -> p a c", p=128)
        xp_v = x_pad.rearrange("(a p) c -> p a c", p=128)
        for a0 in range(0, NGC, GP):
            cnt = min(GP, NGC - a0)
            xt = ppool.tile([128, GP, 32], F32, tag="xt")
            nc.sync.dma_start(xt[:, 0:cnt, :], xg_v[:, a0:a0 + cnt, 0:32])
            ot = ppool.tile([128, GP, 128], BF16, tag="ot")
            # hi into 0:32 and 64:96, lo = x - hi into 32:64, zero 96:128
            nc.vector.tensor_copy(ot[:, 0:cnt, 0:32], xt[:, 0:cnt, :])
            nc.vector.tensor_copy(ot[:, 0:cnt, 64:96], ot[:, 0:cnt, 0:32])
            nc.vector.tensor_tensor(ot[:, 0:cnt, 32:64], xt[:, 0:cnt, :],
                                    ot[:, 0:cnt, 0:32], mybir.AluOpType.subtract)
            nc.vector.memset(ot[:, 0:cnt, 96:128], 0)
            nc.sync.dma_start(xp_v[:, a0:a0 + cnt, :], ot[:, 0:cnt, :])

        # identity for PE transpose
        ident = idp.tile([128, 128], F32)
        nc.vector.memset(ident[:], 1.0)
        nc.gpsimd.affine_select(ident[:], ident[:], pattern=[[-1, 128]],
                                base=0, channel_multiplier=1,
                                compare_op=mybir.AluOpType.is_equal, fill=0.0)
        # xT / bhT: per 128-row chunk of the local shard, transpose [128,36]
        xl_v = xsh.rearrange("(a p) c -> p a c", p=128)
        for a0 in range(0, NCHUNK, GP):
            cnt = min(GP, NCHUNK - a0)
            xa = ppool.tile([128, GP, 36], F32, tag="xa")
            nc.sync.dma_start(xa[:, 0:cnt, :], xl_v[:, a0:a0 + cnt, :])
            tT = ppool.tile([36, GP, 128], F32, tag="tT")
            for s in range(cnt):
                ps = psp.tile([36, 128], F32, tag="pt")
                nc.tensor.transpose(ps[:, :], xa[:, s, :], ident[:])
                nc.vector.tensor_copy(tT[:, s, :], ps[:, :])
            nc.sync.dma_start(
                xT_int.rearrange("c (a f) -> c a f", f=128)[:, a0:a0 + cnt, :],
                tT[0:32, 0:cnt, :])
            nc.sync.dma_start(
                bhT_int.rearrange("c (a f) -> c a f", f=128)[:, a0:a0 + cnt, :],
                tT[32:36, 0:cnt, :])
    _flush(sp=True, swdge=False)

    # ================= TC1: pooling sums + phase-1 convs =================
    with ExitStack() as ctx:
        tc = ctx.enter_context(tile.TileContext(nc))
        wpool = ctx.enter_context(tc.tile_pool(name="wpool", bufs=1))
        gpool = ctx.enter_context(tc.tile_pool(name="gpool", bufs=2))
        gipool = ctx.enter_context(tc.tile_pool(name="gipool", bufs=2))
        sipool = ctx.enter_context(tc.tile_pool(name="sipool", bufs=2))
        cpool = ctx.enter_context(tc.tile_pool(name="cpool", bufs=3))
        stpool = ctx.enter_context(tc.tile_pool(name="stpool", bufs=2))
        pspool = ctx.enter_context(tc.tile_pool(name="pspool", bufs=4,
                                                space="PSUM"))
        pssum = ctx.enter_context(tc.tile_pool(name="pssum", bufs=1,
                                               space="PSUM"))
        mpool = ctx.enter_context(tc.tile_pool(name="mpool", bufs=1))

        # pooling sums: [4,32] = sum over local rows grouped by batch
        xl_t = mpool.tile([128, NCHUNK, 32], F32)
        nc.sync.dma_start(
            xl_t[:], xsh.rearrange("(a p) c -> p a c", p=128)[:, :, 0:32])
        bh_t = mpool.tile([128, NCHUNK, 4], F32)
        nc.sync.dma_start(
            bh_t[:], xsh.rearrange("(a p) c -> p a c", p=128)[:, :, 32:36])
        ps_sums = pssum.tile([4, 32], F32)
        for i in range(NCHUNK):
            nc.tensor.matmul(ps_sums[:, :], bh_t[:, i, :], xl_t[:, i, :],
                             start=(i == 0), stop=(i == NCHUNK - 1))
        sums_sb = mpool.tile([4, 32], F32)
        nc.vector.tensor_copy(sums_sb[:], ps_sums[:])
        nc.sync.dma_start(sums_local[:, :], sums_sb[:])

        # phase-1 convs
        w_sb = wpool.tile([96, WID * KK * 32], BF16)
        nc.sync.dma_start(w_sb[:], wbf[0:96, 0:WID * KK * 32])

        for ji in range(WID):
            for b in range(4):
                key = (ji, b)
                tok = meta1["TOK"][key]
                if tok == 0:
                    continue
                pieces = meta1["pieces"][key]
                scalls = meta1["scalls"][key]
                for (sgs, slen) in meta1["gcalls"][key]:
                    seg_i = sgs // SEG
                    gi_o, _ = offs_g1[(key, sgs)]
                    gi_t = gipool.tile([128, slen // 16], I16, tag="gi")
                    nc.sync.dma_start(gi_t[:],
                                      idxx[:, gi_o:gi_o + slen // 16])
                    g_t = gpool.tile([128, SEG], BF16, tag="g")
                    if not SKIP_GATHER1:
                        nc.gpsimd.dma_gather(
                            g_t[:, 0:slen].rearrange("p (s n) -> p s n", s=1),
                            x_pad[b * BUCK:min((b + 1) * BUCK, NG), :],
                            gi_t[:], num_idxs=slen, num_idxs_reg=slen,
                            elem_size=128, transpose=True, single_packet=False)
                    st_t = stpool.tile([128, 3 * 512], F32, tag="st")
                    nquad = (slen + QUAD - 1) // QUAD
                    for u in range(nquad):
                        if SKIP_MM1:
                            break
                        ps = pspool.tile([128, 512], F32, tag="c4")
                        nc.vector.memset(ps[:], 0)
                        for (seg, uu, band, col0, ts0, ln, k) in pieces:
                            if seg != seg_i or uu != u:
                                continue
                            nc.tensor.matmul(
                                ps[32 * band:32 * band + 32, col0:col0 + ln],
                                w_sb[:, (ji * KK + k) * 32:(ji * KK + k) * 32 + 32],
                                g_t[0:96, ts0:ts0 + ln],
                                start=True, stop=True,
                                tile_position=(0, 32 * band))
                        c4_t = cpool.tile([128, 512], F32, tag="c4sb")
                        nc.vector.tensor_copy(c4_t[:], ps[:])
                        nc.vector.transpose(st_t[:, 512 * u:512 * (u + 1)],
                                            c4_t[:])
                    # scatter calls in this segment
                    for (a, ln) in scalls:
                        if SKIP_SCATTER1 or a // SEG != seg_i:
                            continue
                        si_o, _ = offs_s1[(key, a)]
                        si_t = sipool.tile([128, ln // 16], I16, tag="si")
                        nc.sync.dma_start(si_t[:],
                                          idxx[:, si_o:si_o + ln // 16])
                        aa = a % SEG
                        nc.gpsimd.dma_scatter_add(
                            btabs[ji][:, 0:32],
                            st_t[:, (aa // 128) * 32:((aa + ln) // 128) * 32]
                            .rearrange("p (a c) -> p a c", c=32),
                            si_t[:], num_idxs=ln, num_idxs_reg=ln,
                            elem_size=32, elem_step=64, single_packet=False)

    # ---- allreduce pooling sums ----
    _flush(sp=True, swdge=True)
    if TC1_ONLY:
        nc.finalize()
        if split:
            _split_waits(nc)
        return nc
    nc.gpsimd.collective_compute(
        "AllReduce", mybir.AluOpType.add,
        replica_groups=[list(range(NCORE))],
        ins=[sums_local.ap()], outs=[sums_red.ap()],
    ).then_inc(cc_sem)
    nc.gpsimd.wait_ge(cc_sem, 2)
    nc.all_engine_barrier()

    # ================= TC2: cat assembly =================
    with ExitStack() as ctx:
        tc = ctx.enter_context(tile.TileContext(nc))
        mpool = ctx.enter_context(tc.tile_pool(name="m2", bufs=1))
        bpool = ctx.enter_context(tc.tile_pool(name="b2", bufs=2))
        catp = ctx.enter_context(tc.tile_pool(name="catp", bufs=2))
        ps2 = ctx.enter_context(tc.tile_pool(name="ps2", bufs=3, space="PSUM"))

        pooled_raw = mpool.tile([4, 32], F32)
        nc.sync.dma_start(pooled_raw[:], sums_red[:, :])
        invc_t = mpool.tile([4, 1], F32)
        nc.sync.dma_start(invc_t[:], wf32[0:4, 96:97])
        pooled_sb = mpool.tile([4, 32], F32)
        nc.vector.tensor_scalar_mul(pooled_sb[:], pooled_raw[:], invc_t[:])
        wlin_t = mpool.tile([32, 32], F32)
        nc.sync.dma_start(wlin_t[:], wf32[0:32, 100:132])
        biasb_t = mpool.tile([128, 32], F32)
        nc.sync.dma_start(biasb_t[:], wf32[:, 0:32])
        xT_t = mpool.tile([32, SPAD], F32)
        nc.sync.dma_start(xT_t[:], xT_int[:, :])
        bhT_t = mpool.tile([4, SPAD], F32)
        nc.sync.dma_start(bhT_t[:], bhT_int[:, :])

        GRP = 8
        for blk in range(0, NCHUNK, GRP):
            cnt = min(GRP, NCHUNK - blk)
            cat_t = catp.tile([128, GRP, 256], BF16, tag="cat")
            nc.vector.memset(cat_t[:, 0:cnt, 224:256], 0)
            btiles = []
            for j in range(WID):
                bt = bpool.tile([128, GRP, 128], F32, tag=f"bt{j}")
                nc.sync.dma_start(
                    bt[:, 0:cnt, :],
                    btabs[j].rearrange("(a p h) f -> p a (h f)", p=128, h=2)
                    [:, blk:blk + cnt, :])
                btiles.append(bt)
            for s in range(cnt):
                i = blk + s
                psb = ps2.tile([128, 32], F32, tag="psb0")
                nc.tensor.matmul(psb[:, :], xT_t[:, 128 * i:128 * (i + 1)],
                                 wlin_t[:, :], start=True, stop=True)
                nc.vector.tensor_tensor(cat_t[:, s, 0:32], psb[:, :],
                                        biasb_t[:, :], mybir.AluOpType.add)
                psp = ps2.tile([128, 32], F32, tag="psp")
                nc.tensor.matmul(psp[:, :], bhT_t[:, 128 * i:128 * (i + 1)],
                                 pooled_sb[:, :], start=True, stop=True)
                nc.vector.tensor_copy(cat_t[:, s, 192:224], psp[:, :])
                for j in range(WID):
                    nc.vector.tensor_tensor(
                        cat_t[:, s, 32 * (j + 1):32 * (j + 2)],
                        btiles[j][:, s, 0:32], btiles[j][:, s, 64:96],
                        mybir.AluOpType.add)
            nc.sync.dma_start(
                cat_local.rearrange("(a p) f -> p a f", p=128)
                [:, blk:blk + cnt, :],
                cat_t[:, 0:cnt, :])

    # ---- allgather cat ----
    _flush(sp=True, swdge=False)
    nc.gpsimd.collective_compute(
        "AllGather", mybir.AluOpType.bypass,
        replica_groups=[list(range(NCORE))],
        ins=[cat_local.ap()], outs=[catg.ap()],
    ).then_inc(cc_sem)
    nc.gpsimd.wait_ge(cc_sem, 3)
    nc.all_engine_barrier()

    # ================= TC3: phase-2 output conv =================
    with ExitStack() as ctx:
        tc = ctx.enter_context(tile.TileContext(nc))
        wpool = ctx.enter_context(tc.tile_pool(name="w3", bufs=1))
        gpool = ctx.enter_context(tc.tile_pool(name="g3", bufs=2))
        gipool = ctx.enter_context(tc.tile_pool(name="gi3", bufs=2))
        sipool = ctx.enter_context(tc.tile_pool(name="si3", bufs=2))
        cpool = ctx.enter_context(tc.tile_pool(name="c3", bufs=3))
        stpool = ctx.enter_context(tc.tile_pool(name="st3", bufs=2))
        pspool = ctx.enter_context(tc.tile_pool(name="ps3", bufs=6,
                                                space="PSUM"))
        wa_sb = wpool.tile([128, KK * 32], BF16)
        nc.sync.dma_start(wa_sb[:], wbf[:, 4320:4320 + KK * 32])
        wb_sb = wpool.tile([96, KK * 32], BF16)
        nc.sync.dma_start(wb_sb[:], wbf[0:96, 5184:5184 + KK * 32])

        for b in range(4):
            key = (0, b)
            tok = meta2["TOK"][key]
            if tok == 0:
                continue
            pieces = meta2["pieces"][key]
            scalls = meta2["scalls"][key]
            for (sgs, slen) in meta2["gcalls"][key]:
                seg_i = sgs // SEG
                gi_o, _ = offs_g2[(key, sgs)]
                gi_t = gipool.tile([128, slen // 16], I16, tag="gi")
                nc.sync.dma_start(gi_t[:], idxx[:, gi_o:gi_o + slen // 16])
                g_t = gpool.tile([128, 2, slen], BF16, tag="g")
                nc.gpsimd.dma_gather(
                    g_t[:, :, :],
                    catg[b * BUCK:min((b + 1) * BUCK, NG), :],
                    gi_t[:], num_idxs=slen, num_idxs_reg=slen,
                    elem_size=256, transpose=True, single_packet=False)
                st_t = stpool.tile([128, 3 * 512], F32, tag="st")
                nquad = (slen + QUAD - 1) // QUAD
                for u in range(nquad):
                    ps = pspool.tile([128, 512], F32, tag="c4")
                    nc.vector.memset(ps[:], 0)
                    for (seg, uu, band, col0, ts0, ln, k) in pieces:
                        if seg != seg_i or uu != u:
                            continue
                        nc.tensor.matmul(
                            ps[32 * band:32 * band + 32, col0:col0 + ln],
                            wa_sb[:, k * 32:k * 32 + 32],
                            g_t[:, 0, ts0:ts0 + ln],
                            start=True, stop=False,
                            tile_position=(0, 32 * band))
                        nc.tensor.matmul(
                            ps[32 * band:32 * band + 32, col0:col0 + ln],
                            wb_sb[:, k * 32:k * 32 + 32],
                            g_t[0:96, 1, ts0:ts0 + ln],
                            start=False, stop=True,
                            tile_position=(0, 32 * band))
                    c4_t = cpool.tile([128, 512], F32, tag="c4sb")
                    nc.vector.tensor_copy(c4_t[:], ps[:])
                    nc.vector.transpose(st_t[:, 512 * u:512 * (u + 1)],
                                        c4_t[:])
                for (a, ln) in scalls:
                    if a // SEG != seg_i:
                        continue
                    si_o, _ = offs_s2[(key, a)]
                    si_t = sipool.tile([128, ln // 16], I16, tag="si")
                    nc.sync.dma_start(si_t[:], idxx[:, si_o:si_o + ln // 16])
                    aa = a % SEG
                    nc.gpsimd.dma_scatter_add(
                        ytab[:, 0:32],
                        st_t[:, (aa // 128) * 32:((aa + ln) // 128) * 32]
                        .rearrange("p (a c) -> p a c", c=32),
                        si_t[:], num_idxs=ln, num_idxs_reg=ln,
                        elem_size=32, elem_step=64, single_packet=False)

    _flush(sp=False, swdge=True)  # ytab scatters before TC4 reads

    # ================= TC4: BN + ReLU + output =================
    with ExitStack() as ctx:
        tc = ctx.enter_context(tile.TileContext(nc))
        mpool = ctx.enter_context(tc.tile_pool(name="m4", bufs=1))
        ypool = ctx.enter_context(tc.tile_pool(name="y4", bufs=2))
        bns_t = mpool.tile([128, 32], F32)
        nc.sync.dma_start(bns_t[:], wf32[:, 32:64])
        bnt_t = mpool.tile([128, 32], F32)
        nc.sync.dma_start(bnt_t[:], wf32[:, 64:96])
        GRP = 8
        for blk in range(0, NCHUNK, GRP):
            cnt = min(GRP, NCHUNK - blk)
            yt = ypool.tile([128, GRP, 128], F32, tag="yt")
            nc.sync.dma_start(
                yt[:, 0:cnt, :],
                ytab.rearrange("(a p h) f -> p a (h f)", p=128, h=2)
                [:, blk:blk + cnt, :])
            yo = ypool.tile([128, GRP, 32], F32, tag="yo")
            for s in range(cnt):
                nc.vector.tensor_tensor(yo[:, s, :], yt[:, s, 0:32],
                                        yt[:, s, 64:96], mybir.AluOpType.add)
                nc.vector.tensor_tensor(yo[:, s, :], yo[:, s, :],
                                        bns_t[:, :], mybir.AluOpType.mult)
                nc.vector.tensor_tensor(yo[:, s, :], yo[:, s, :],
                                        bnt_t[:, :], mybir.AluOpType.add)
                nc.vector.tensor_scalar_max(yo[:, s, :], yo[:, s, :], 0.0)
            nc.sync.dma_start(
                y_out.rearrange("(a p) c -> p a c", p=128)[:, blk:blk + cnt, :],
                yo[:, 0:cnt, :])

    nc.finalize()
    if split:
        _split_waits(nc)
    return nc


def _split_waits(nc, maxw=1):
    cnt = 0
    for fn in nc.m.functions:
        for bb in fn.blocks:
            insts = list(bb.instructions)
            newlist = []
            changed = False
            for ins in insts:
                si = ins.sync_info
                if si is not None and si.on_wait and len(si.on_wait) > maxw:
                    waits = list(si.on_wait)
                    extra, keep = waits[:-maxw], waits[-maxw:]
                    while extra:
                        chunk, extra = extra[:maxw], extra[maxw:]
                        cnt += 1
                        newlist.append(mybir.InstNoOp(
                            name=f"waitsplit-{cnt}",
                            engine=ins.engine,
                            sync_info=mybir.SyncInfo(on_wait=chunk,
                                                     on_update=[]),
                            bass_nofuse=True,
                        ))
                    si.on_wait = keep
                    changed = True
                newlist.append(ins)
            if changed:
                bb.instructions = newlist
    return cnt


def make_program(inputs, split=True):
    x = np.asarray(inputs["x"], np.float32)
    batch_idx = np.asarray(inputs["batch_idx"], np.int32)
    in_maps = np.asarray(inputs["in_maps"], np.int64)
    out_maps = np.asarray(inputs["out_maps"], np.int64)
    W_lin = np.asarray(inputs["W_lin"], np.float32)
    b_lin = np.asarray(inputs["b_lin"], np.float32)
    W_branch = np.asarray(inputs["W_branch"], np.float32)
    W_out = np.asarray(inputs["W_out"], np.float32)
    gamma = np.asarray(inputs["gamma"], np.float32)
    beta = np.asarray(inputs["beta"], np.float32)
    bn_mean = np.asarray(inputs["bn_mean"], np.float32)
    bn_var = np.asarray(inputs["bn_var"], np.float32)

    # ---- host prep: packed weight tensors ----
    wbf = np.zeros((128, 6048), ml_dtypes.bfloat16)
    for j in range(WID):
        for k in range(KK):
            w_hi, w_lo = _hi_lo(W_branch[j, k])
            col = (j * KK + k) * 32
            wbf[0:32, col:col + 32] = w_hi
            wbf[32:64, col:col + 32] = w_hi
            wbf[64:96, col:col + 32] = w_lo
    for k in range(KK):
        wk = W_out[k].astype(ml_dtypes.bfloat16)
        wbf[:, 4320 + k * 32:4320 + k * 32 + 32] = wk[0:128]
        wbf[0:96, 5184 + k * 32:5184 + k * 32 + 32] = wk[128:224]

    counts = np.bincount(batch_idx, minlength=B).astype(np.float32)
    counts[counts == 0] = 1.0
    s_bn = (gamma / np.sqrt(bn_var + EPS)).astype(np.float32)
    t_bn = (beta - bn_mean * s_bn).astype(np.float32)
    wf32 = np.zeros((128, 132), np.float32)
    wf32[:, 0:32] = b_lin.reshape(1, 32)
    wf32[:, 32:64] = s_bn.reshape(1, 32)
    wf32[:, 64:96] = t_bn.reshape(1, 32)
    wf32[0:4, 96] = 1.0 / counts
    wf32[0:32, 100:132] = W_lin

    # streams (both phases gather from the SPAD-padded global row space)
    remap = ((np.arange(N) // S) * SPAD + (np.arange(N) % S)).astype(np.int64)
    meta1, pc1 = _prep_streams(in_maps, out_maps, list(range(WID)), NG,
                               remap_in=remap)
    meta2, pc2 = _prep_streams(in_maps, out_maps, [WID], NG, remap_in=remap)
    arrs, offs_list, wall = _pack_idx([(meta1, pc1), (meta2, pc2)])
    (offs_g1, offs_s1), (offs_g2, offs_s2) = offs_list

    nc = _build(meta1, meta2, wall,
                (offs_g1, offs_s1, offs_g2, offs_s2), split=split)

    in_maps_percore = []
    for c in range(NCORE):
        xa = np.zeros((SPAD, 36), np.float32)
        xa[0:S, 0:32] = x[c * S:(c + 1) * S]
        xa[np.arange(S), 32 + batch_idx[c * S:(c + 1) * S]] = 1.0
        in_maps_percore.append({
            "xsh": xa, "idxc": arrs[c], "wbf": wbf, "wf32": wf32,
        })

    global _last_meta
    _last_meta = (meta1, meta2)
    return nc, in_maps_percore


def kernel(**inputs):
    nc, in_maps_percore = make_program(inputs, split=True)
    res = run_bass_kernel_spmd(nc, in_maps_percore, list(range(NCORE)))
    global _last_res
    _last_res = res
    y = np.concatenate([res.results[c]["y_out"][0:S] for c in range(NCORE)],
                       axis=0)
    return y.astype(np.float32)


_last_res = None
_last_meta = None


if __name__ == "__main__":
    pass
